# revision 13
# baseline (speedup 1.0000x reference)
"""GroupFC kernel for Trainium2, data-parallel across 8 NeuronCores.

Problem: out = data @ W.T + b
  data: [32768, 1024] f32, W: [1024, 1024] f32, b: [1024] f32

Strategy (v2):
  - Shard batch dim across 8 cores (4096 rows each); replicate W, b.
  - Transposed-output formulation: outT[o, b] = sum_k W[o,k] d[b,k] + b[o].
    Stationary operand = W tiles (out-dim on PSUM partitions), moving
    operand = data columns (batch on the free dim).
  - Mixed precision along the contraction: k-blocks 0..5 in bf16
    (1 col/cycle), k-blocks 6..7 quantized to fp8-e4m3 and run as
    DoubleRow matmuls (measured ~2x column rate). Host-measured rel err
    of this split is ~1.5e-2 (budget 2e-2).
  - All W values pre-scaled by 128 on the host so the fp8 weights avoid
    the e4m3 subnormal range; the single fused evacuation applies 1/128
    and the per-out-row bias in one pass (ACT for one PSUM bank, DVE for
    the other), emitting bf16.
  - Host post-pass transposes outT back to [batch, out] f32.
"""

import sys
from contextlib import ExitStack

import numpy as np

try:
    import concourse.bass as bass  # noqa: F401
except ImportError:
    sys.path.insert(0, "/opt/trn_rl_repo")

import ml_dtypes

import concourse.tile as tile
from concourse import bacc, mybir
from concourse.bass_utils import run_bass_kernel_spmd

N_CORES = 8
BATCH = 32768
SHARD = BATCH // N_CORES  # 4096
IN_DIM = 1024
OUT_DIM = 1024
P = 128
KB = 6  # bf16 k-blocks (0..768); the last 2 blocks (768..1024) go fp8
NQ = 4  # batch quarters per core (1024 columns each)
QCOL = SHARD // NQ  # 1024
NO = OUT_DIM // P  # 8 output-row blocks
SCALE = 128.0
E4 = ml_dtypes.float8_e4m3
BF = ml_dtypes.bfloat16

_CACHE = {}


def _build():
    nc = bacc.Bacc("TRN2", target_bir_lowering=False, debug=False)
    dT = nc.dram_tensor(
        "dT", [KB, P, SHARD], mybir.dt.bfloat16, kind="ExternalInput"
    ).ap()
    d8a = nc.dram_tensor(
        "d8a", [2, P, 2, QCOL], mybir.dt.float8e4, kind="ExternalInput"
    ).ap()
    d8b = nc.dram_tensor(
        "d8b", [2, P, 4, QCOL], mybir.dt.float8e4, kind="ExternalInput"
    ).ap()
    wT = nc.dram_tensor(
        "wT", [KB, P, OUT_DIM], mybir.dt.bfloat16, kind="ExternalInput"
    ).ap()
    w8hi = nc.dram_tensor(
        "w8hi", [P, 2, OUT_DIM], mybir.dt.float8e4, kind="ExternalInput"
    ).ap()
    w8lo = nc.dram_tensor(
        "w8lo", [P, 2, OUT_DIM], mybir.dt.float8e4, kind="ExternalInput"
    ).ap()
    biasb = nc.dram_tensor(
        "biasb", [P, NO], mybir.dt.float32, kind="ExternalInput"
    ).ap()
    wmup = nc.dram_tensor(
        "wmup", [P, 256], mybir.dt.bfloat16, kind="ExternalInput"
    ).ap()
    outT = nc.dram_tensor(
        "outT", [OUT_DIM, SHARD], mybir.dt.bfloat16, kind="ExternalOutput"
    ).ap()

    with tile.TileContext(nc) as tc:
        with ExitStack() as ctx:
            wp = ctx.enter_context(tc.tile_pool(name="w", bufs=1))
            dp = ctx.enter_context(tc.tile_pool(name="d", bufs=1))
            bp = ctx.enter_context(tc.tile_pool(name="misc", bufs=1))
            pp = ctx.enter_context(tc.tile_pool(name="psum", bufs=4, space="PSUM"))
            op = ctx.enter_context(tc.tile_pool(name="o", bufs=8))

            w_t = [None] * KB
            d_t = [[None] * NQ for _ in range(KB)]
            d8_t = [None] * NQ
            w8hi_t = None
            w8lo_t = None
            bias_t = None
            wmup_t = None

            # Load plan: tiny warmup + bias first, then the fp8 weights/data
            # for q0 (512 KiB unlocks the DR-first matmuls of the first four
            # groups), then (wT[k], dT[k] q0) pairs in consumption order, then
            # the remaining quarters. Alternate the two HWDGE rings.
            loads = [("wm", 0, 0), ("bias", 0, 0), ("w8hi", 0, 0), ("d8", 0, 0)]
            for k in range(KB):
                loads.append(("w", k, 0))
                loads.append(("d", k, 0))
            for q in range(1, NQ):
                for k in range(KB if q < 2 else KB - 2):
                    loads.append(("d", k, q))
                loads.append(("d8", 0, q))
                if q == 1:
                    # fp8 weights for k-blocks 4,5: first consumed by the
                    # q2 groups, far past the critical load ramp.
                    loads.append(("w8lo", 0, 0))

            for i, (kind, k, q) in enumerate(loads):
                eng = nc.scalar if i % 2 == 0 else nc.sync
                if kind == "wm":
                    wmup_t = bp.tile([P, 256], mybir.dt.bfloat16, tag="wm", name="wmup_t")
                    eng.dma_start(out=wmup_t[:], in_=wmup[:, :])
                elif kind == "w":
                    w_t[k] = wp.tile([P, OUT_DIM], mybir.dt.bfloat16, tag=f"w{k}", name=f"w_t{k}")
                    eng.dma_start(out=w_t[k][:], in_=wT[k, :, :])
                elif kind == "d":
                    d_t[k][q] = dp.tile([P, QCOL], mybir.dt.bfloat16, tag=f"d{k}_{q}", name=f"d_t{k}_{q}")
                    eng.dma_start(
                        out=d_t[k][q][:], in_=dT[k, :, q * QCOL : (q + 1) * QCOL]
                    )
                elif kind == "w8hi":
                    w8hi_t = wp.tile([P, 2, OUT_DIM], mybir.dt.float8e4, tag="w8hi", name="w8hi_t")
                    eng.dma_start(out=w8hi_t[:], in_=w8hi[:, :, :])
                elif kind == "w8lo":
                    w8lo_t = wp.tile([P, 2, OUT_DIM], mybir.dt.float8e4, tag="w8lo", name="w8lo_t")
                    eng.dma_start(out=w8lo_t[:], in_=w8lo[:, :, :])
                elif kind == "d8":
                    nblk = 2 if q < 2 else 4
                    d8_t[q] = dp.tile([P, nblk, QCOL], mybir.dt.float8e4, tag=f"d8_{q}", name=f"d8_t{q}")
                    src = d8a[q] if q < 2 else d8b[q - 2]
                    eng.dma_start(out=d8_t[q][:], in_=src[:, :, :])
                else:
                    bias_t = bp.tile([P, NO], mybir.dt.float32, tag="bias", name="bias_t")
                    eng.dma_start(out=bias_t[:], in_=biasb[:, :])

            # Warmup: get the HAM clock ramping while the first real tiles
            # stream in. Gated on the (tiny, first-in-queue) wmup DMA so the
            # PE's first activity never precedes the first useful DMA.
            ps_first = [
                pp.tile([P, 512], mybir.dt.float32, tag="pa", name="ps_a0"),
                pp.tile([P, 512], mybir.dt.float32, tag="pb", name="ps_b0"),
            ]
            for i in range(18):
                nc.tensor.matmul(
                    ps_first[0][:, 0:256], wmup_t[:, 0:P], wmup_t[:],
                    start=True, stop=True, skip_group_check=True,
                )

            dr = mybir.MatmulPerfMode.DoubleRow

            def emit_dr(psA, psB, q, o, first):
                # When the DR matmuls open a bank's accumulation (first=True),
                # only the FIRST matmul per bank may set start=True: start
                # clears has_written for the WHOLE bank, so a second start on
                # the other half would wipe the first half's result. The
                # second matmul (start=False) overwrites its half because its
                # has_written bits are clear.
                # q0/q1 run one fp8 pair (k-blocks 6,7); q2/q3 run two pairs
                # (4,5 then 6,7) -- half the batch at a deeper fp8 split.
                osl = slice(o * P, (o + 1) * P)
                npair = 1 if q < 2 else 2
                for gi in range(npair):
                    if q < 2:
                        wsl = w8hi_t[:, :, osl]
                        dsl = d8_t[q]
                        dlo = 0
                    else:
                        wsl = (w8lo_t if gi == 0 else w8hi_t)[:, :, osl]
                        dsl = d8_t[q]
                        dlo = 2 * gi
                    last = gi == npair - 1
                    st = first and gi == 0
                    nc.tensor.matmul(
                        psA[:, 0:256], wsl, dsl[:, dlo : dlo + 2, 0:256],
                        start=st, stop=(not first) and last, perf_mode=dr,
                        skip_group_check=True,
                    )
                    nc.tensor.matmul(
                        psA[:, 256:512], wsl, dsl[:, dlo : dlo + 2, 256:512],
                        start=False, stop=(not first) and last, perf_mode=dr,
                        skip_group_check=True,
                    )
                    nc.tensor.matmul(
                        psB[:, 0:256], wsl, dsl[:, dlo : dlo + 2, 512:768],
                        start=st, stop=(not first) and last, perf_mode=dr,
                        skip_group_check=True,
                    )
                    nc.tensor.matmul(
                        psB[:, 256:512], wsl, dsl[:, dlo : dlo + 2, 768:QCOL],
                        start=False, stop=(not first) and last, perf_mode=dr,
                        skip_group_check=True,
                    )

            def emit_evac(psA, psB, q, o):
                # Fused evacuation: out = psum/128 + bias[o], to bf16.
                # Both banks on DVE: the scalar/sync queues stay dedicated
                # to load DMAs so PSUM recycling never stalls behind them.
                # Separate half-tiles so each store only waits on its own
                # evacuation and the two stores ride both HW rings in
                # parallel (loads are all enqueued already, so a store's
                # semaphore wait cannot delay any load).
                osl = slice(o * P, (o + 1) * P)
                bcol = bias_t[:, o : o + 1]
                c0 = q * QCOL
                osbA = op.tile([P, 512], mybir.dt.bfloat16, tag="osbA", name="osbA")
                nc.vector.tensor_scalar(
                    osbA[:], psA[:],
                    1.0 / SCALE, bcol,
                    mybir.AluOpType.mult, mybir.AluOpType.add,
                )
                nc.scalar.dma_start(out=outT[osl, c0 : c0 + 512], in_=osbA[:])
                osbB = op.tile([P, 512], mybir.dt.bfloat16, tag="osbB", name="osbB")
                nc.vector.tensor_scalar(
                    osbB[:], psB[:],
                    1.0 / SCALE, bcol,
                    mybir.AluOpType.mult, mybir.AluOpType.add,
                )
                nc.sync.dma_start(out=outT[osl, c0 + 512 : c0 + QCOL], in_=osbB[:])

            # Phase 1 — groups (q0, o=0..3), DR-first: their fp8 matmuls only
            # need w8+d8q0 (512 KiB), so the PE does real work while the bf16
            # weight/data tiles stream in; the bf16 part then runs k-outer
            # across the four groups, matching DMA arrival order.
            ph1 = []
            for o in range(4):
                psA, psB = ps_first if o == 0 else (
                    pp.tile([P, 512], mybir.dt.float32, tag="pa", name="psA"),
                    pp.tile([P, 512], mybir.dt.float32, tag="pb", name="psB"),
                )
                ph1.append((psA, psB))
                emit_dr(psA, psB, 0, o, first=True)
            for k in range(KB):
                for o in range(4):
                    psA, psB = ph1[o]
                    lhsT = w_t[k][:, o * P : (o + 1) * P]
                    nc.tensor.matmul(
                        psA[:], lhsT, d_t[k][0][:, 0:512],
                        start=False, stop=(k == KB - 1),
                    )
                    nc.tensor.matmul(
                        psB[:], lhsT, d_t[k][0][:, 512:QCOL],
                        start=False, stop=(k == KB - 1),
                    )
            for o in range(4):
                emit_evac(ph1[o][0], ph1[o][1], 0, o)

            # Phase 2 — everything else in normal order (bf16 k-major, DR
            # tail) since all operands are SBUF-resident by then.
            for q in range(NQ):
                for o in range(4 if q == 0 else 0, NO):
                    psA = pp.tile([P, 512], mybir.dt.float32, tag="pa", name="psA")
                    psB = pp.tile([P, 512], mybir.dt.float32, tag="pb", name="psB")
                    for k in range(KB if q < 2 else KB - 2):
                        lhsT = w_t[k][:, o * P : (o + 1) * P]
                        nc.tensor.matmul(
                            psA[:], lhsT, d_t[k][q][:, 0:512],
                            start=(k == 0), stop=False,
                        )
                        nc.tensor.matmul(
                            psB[:], lhsT, d_t[k][q][:, 512:QCOL],
                            start=(k == 0), stop=False,
                        )
                    emit_dr(psA, psB, q, o, first=False)
                    emit_evac(psA, psB, q, o)

    nc.compile()
    return nc


def _get_nc():
    if "nc" not in _CACHE:
        _CACHE["nc"] = _build()
    return _CACHE["nc"]


def _prep_weights(W, b):
    W = np.asarray(W, dtype=np.float32)
    b = np.asarray(b, dtype=np.float32)
    Ws = W * SCALE
    # wT[k, p, o] = W[o, k*128+p] * 128  (bf16)
    wT = np.ascontiguousarray(
        Ws[:, : KB * P].T.reshape(KB, P, OUT_DIM).astype(BF)
    )
    # w8lo[p, i, o] = e4m3(W[o, 512 + i*128 + p] * 128)  (k-blocks 4,5)
    # w8hi[p, i, o] = e4m3(W[o, 768 + i*128 + p] * 128)  (k-blocks 6,7)
    w8lo = np.ascontiguousarray(
        Ws[:, 4 * P : 6 * P].T.reshape(2, P, OUT_DIM).transpose(1, 0, 2).astype(E4)
    )
    w8hi = np.ascontiguousarray(
        Ws[:, 6 * P :].T.reshape(2, P, OUT_DIM).transpose(1, 0, 2).astype(E4)
    )
    bias2 = np.ascontiguousarray(b.reshape(NO, P).T)  # [128, 8] f32
    wmup = np.zeros((P, 256), dtype=BF)
    return wT, w8lo, w8hi, bias2, wmup


def _prep_inputs(data, W, b):
    data = np.asarray(data, dtype=np.float32)
    wT, w8lo, w8hi, bias2, wmup = _prep_weights(W, b)
    in_maps = []
    for c in range(N_CORES):
        shard = data[c * SHARD : (c + 1) * SHARD]  # [4096, 1024] f32
        # dT[k, p, b] = bf16(shard[b, k*128+p])
        dTc = np.ascontiguousarray(
            shard[:, : KB * P].T.reshape(KB, P, SHARD).astype(BF)
        )
        # d8a[q, p, i, j] = e4m3(shard[q*1024+j, 768 + i*128 + p]), q = 0,1
        d8at = shard[: 2 * QCOL, 6 * P :].T.reshape(2, P, 2, QCOL)
        d8ac = np.ascontiguousarray(d8at.transpose(2, 1, 0, 3).astype(E4))
        # d8b[q, p, i, j] = e4m3(shard[(q+2)*1024+j, 512 + i*128 + p]), q = 0,1
        d8bt = shard[2 * QCOL :, 4 * P :].T.reshape(4, P, 2, QCOL)
        d8bc = np.ascontiguousarray(d8bt.transpose(2, 1, 0, 3).astype(E4))
        in_maps.append(
            {"dT": dTc, "d8a": d8ac, "d8b": d8bc, "wT": wT, "w8lo": w8lo,
             "w8hi": w8hi, "biasb": bias2, "wmup": wmup}
        )
    return in_maps


def _run(data, W, b, trace=False, **trace_kw):
    nc = _get_nc()
    in_maps = _prep_inputs(data, W, b)
    res = run_bass_kernel_spmd(
        nc, in_maps, list(range(N_CORES)), trace=trace, **trace_kw
    )
    out = np.concatenate(
        [
            np.asarray(res.results[c]["outT"]).T.astype(np.float32)
            for c in range(N_CORES)
        ],
        axis=0,
    )
    return out, res


def kernel(**inputs) -> np.ndarray:
    out, _ = _run(inputs["data"], inputs["W"], inputs["b"])
    return out


# revision 14
# speedup vs baseline: 1.0099x; 1.0099x over previous
"""GroupFC kernel for Trainium2, data-parallel across 8 NeuronCores.

Problem: out = data @ W.T + b
  data: [32768, 1024] f32, W: [1024, 1024] f32, b: [1024] f32

Strategy (v2):
  - Shard batch dim across 8 cores (4096 rows each); replicate W, b.
  - Transposed-output formulation: outT[o, b] = sum_k W[o,k] d[b,k] + b[o].
    Stationary operand = W tiles (out-dim on PSUM partitions), moving
    operand = data columns (batch on the free dim).
  - Mixed precision along the contraction: k-blocks 0..5 in bf16
    (1 col/cycle), k-blocks 6..7 quantized to fp8-e4m3 and run as
    DoubleRow matmuls (measured ~2x column rate). Host-measured rel err
    of this split is ~1.5e-2 (budget 2e-2).
  - All W values pre-scaled by 128 on the host so the fp8 weights avoid
    the e4m3 subnormal range; the single fused evacuation applies 1/128
    and the per-out-row bias in one pass (ACT for one PSUM bank, DVE for
    the other), emitting bf16.
  - Host post-pass transposes outT back to [batch, out] f32.
"""

import sys
from contextlib import ExitStack

import numpy as np

try:
    import concourse.bass as bass  # noqa: F401
except ImportError:
    sys.path.insert(0, "/opt/trn_rl_repo")

import ml_dtypes

import concourse.tile as tile
from concourse import bacc, mybir
from concourse.bass_utils import run_bass_kernel_spmd

N_CORES = 8
BATCH = 32768
SHARD = BATCH // N_CORES  # 4096
IN_DIM = 1024
OUT_DIM = 1024
P = 128
KB = 6  # bf16 k-blocks (0..768); the last 2 blocks (768..1024) go fp8
NQ = 4  # batch quarters per core (1024 columns each)
QCOL = SHARD // NQ  # 1024
NO = OUT_DIM // P  # 8 output-row blocks
SCALE = 128.0
E4 = ml_dtypes.float8_e4m3
BF = ml_dtypes.bfloat16

_CACHE = {}


def _build():
    nc = bacc.Bacc("TRN2", target_bir_lowering=False, debug=False)
    dT = nc.dram_tensor(
        "dT", [KB, P, SHARD], mybir.dt.bfloat16, kind="ExternalInput"
    ).ap()
    d8a = nc.dram_tensor(
        "d8a", [2, P, 2, QCOL], mybir.dt.float8e4, kind="ExternalInput"
    ).ap()
    d8b = nc.dram_tensor(
        "d8b", [2, P, 4, QCOL], mybir.dt.float8e4, kind="ExternalInput"
    ).ap()
    wT = nc.dram_tensor(
        "wT", [KB, P, OUT_DIM], mybir.dt.bfloat16, kind="ExternalInput"
    ).ap()
    w8hi = nc.dram_tensor(
        "w8hi", [P, 2, OUT_DIM], mybir.dt.float8e4, kind="ExternalInput"
    ).ap()
    w8lo = nc.dram_tensor(
        "w8lo", [P, 2, OUT_DIM], mybir.dt.float8e4, kind="ExternalInput"
    ).ap()
    biasb = nc.dram_tensor(
        "biasb", [P, NO], mybir.dt.float32, kind="ExternalInput"
    ).ap()
    wmup = nc.dram_tensor(
        "wmup", [P, 256], mybir.dt.bfloat16, kind="ExternalInput"
    ).ap()
    outT = nc.dram_tensor(
        "outT", [OUT_DIM, SHARD], mybir.dt.bfloat16, kind="ExternalOutput"
    ).ap()

    with tile.TileContext(nc) as tc:
        with ExitStack() as ctx:
            wp = ctx.enter_context(tc.tile_pool(name="w", bufs=1))
            dp = ctx.enter_context(tc.tile_pool(name="d", bufs=1))
            bp = ctx.enter_context(tc.tile_pool(name="misc", bufs=1))
            pp = ctx.enter_context(tc.tile_pool(name="psum", bufs=4, space="PSUM"))
            op = ctx.enter_context(tc.tile_pool(name="o", bufs=8))

            w_t = [None] * KB
            d_t = [[None] * NQ for _ in range(KB)]
            d8_t = [None] * NQ
            w8hi_t = None
            w8lo_t = None
            bias_t = None
            wmup_t = None

            # Load plan: tiny warmup + bias first, then the fp8 weights/data
            # for q0 (512 KiB unlocks the DR-first matmuls of the first four
            # groups), then (wT[k], dT[k] q0) pairs in consumption order, then
            # the remaining quarters. Alternate the two HWDGE rings.
            loads = [("wm", 0, 0), ("bias", 0, 0), ("w8hi", 0, 0), ("d8", 0, 0)]
            for k in range(KB):
                loads.append(("w", k, 0))
                loads.append(("d", k, 0))
            for q in range(1, NQ):
                for k in range(KB if q < 2 else KB - 2):
                    loads.append(("d", k, q))
                loads.append(("d8", 0, q))
                if q == 1:
                    # fp8 weights for k-blocks 4,5: first consumed by the
                    # q2 groups, far past the critical load ramp.
                    loads.append(("w8lo", 0, 0))

            for i, (kind, k, q) in enumerate(loads):
                eng = nc.scalar if i % 2 == 0 else nc.sync
                if kind == "wm":
                    wmup_t = bp.tile([P, 256], mybir.dt.bfloat16, tag="wm", name="wmup_t")
                    eng.dma_start(out=wmup_t[:], in_=wmup[:, :])
                elif kind == "w":
                    w_t[k] = wp.tile([P, OUT_DIM], mybir.dt.bfloat16, tag=f"w{k}", name=f"w_t{k}")
                    eng.dma_start(out=w_t[k][:], in_=wT[k, :, :])
                elif kind == "d":
                    d_t[k][q] = dp.tile([P, QCOL], mybir.dt.bfloat16, tag=f"d{k}_{q}", name=f"d_t{k}_{q}")
                    eng.dma_start(
                        out=d_t[k][q][:], in_=dT[k, :, q * QCOL : (q + 1) * QCOL]
                    )
                elif kind == "w8hi":
                    w8hi_t = wp.tile([P, 2, OUT_DIM], mybir.dt.float8e4, tag="w8hi", name="w8hi_t")
                    eng.dma_start(out=w8hi_t[:], in_=w8hi[:, :, :])
                elif kind == "w8lo":
                    w8lo_t = wp.tile([P, 2, OUT_DIM], mybir.dt.float8e4, tag="w8lo", name="w8lo_t")
                    eng.dma_start(out=w8lo_t[:], in_=w8lo[:, :, :])
                elif kind == "d8":
                    nblk = 2 if q < 2 else 4
                    d8_t[q] = dp.tile([P, nblk, QCOL], mybir.dt.float8e4, tag=f"d8_{q}", name=f"d8_t{q}")
                    src = d8a[q] if q < 2 else d8b[q - 2]
                    eng.dma_start(out=d8_t[q][:], in_=src[:, :, :])
                else:
                    bias_t = bp.tile([P, NO], mybir.dt.float32, tag="bias", name="bias_t")
                    eng.dma_start(out=bias_t[:], in_=biasb[:, :])

            # Warmup: get the HAM clock ramping while the first real tiles
            # stream in. Gated on an on-chip memset so it starts as soon as
            # the engines come up, independent of DMA latency.
            scr = bp.tile([P, 256], mybir.dt.bfloat16, tag="scr", name="scr")
            nc.vector.memset(scr[:], 0)
            ps_first = [
                pp.tile([P, 512], mybir.dt.float32, tag="pa", name="ps_a0"),
                pp.tile([P, 512], mybir.dt.float32, tag="pb", name="ps_b0"),
            ]
            for i in range(18):
                nc.tensor.matmul(
                    ps_first[0][:, 0:256], scr[:, 0:P], scr[:],
                    start=True, stop=True, skip_group_check=True,
                )

            dr = mybir.MatmulPerfMode.DoubleRow

            def emit_dr(psA, psB, q, o, first):
                # When the DR matmuls open a bank's accumulation (first=True),
                # only the FIRST matmul per bank may set start=True: start
                # clears has_written for the WHOLE bank, so a second start on
                # the other half would wipe the first half's result. The
                # second matmul (start=False) overwrites its half because its
                # has_written bits are clear.
                # q0/q1 run one fp8 pair (k-blocks 6,7); q2/q3 run two pairs
                # (4,5 then 6,7) -- half the batch at a deeper fp8 split.
                osl = slice(o * P, (o + 1) * P)
                npair = 1 if q < 2 else 2
                for gi in range(npair):
                    if q < 2:
                        wsl = w8hi_t[:, :, osl]
                        dsl = d8_t[q]
                        dlo = 0
                    else:
                        wsl = (w8lo_t if gi == 0 else w8hi_t)[:, :, osl]
                        dsl = d8_t[q]
                        dlo = 2 * gi
                    last = gi == npair - 1
                    st = first and gi == 0
                    nc.tensor.matmul(
                        psA[:, 0:256], wsl, dsl[:, dlo : dlo + 2, 0:256],
                        start=st, stop=(not first) and last, perf_mode=dr,
                        skip_group_check=True,
                    )
                    nc.tensor.matmul(
                        psA[:, 256:512], wsl, dsl[:, dlo : dlo + 2, 256:512],
                        start=False, stop=(not first) and last, perf_mode=dr,
                        skip_group_check=True,
                    )
                    nc.tensor.matmul(
                        psB[:, 0:256], wsl, dsl[:, dlo : dlo + 2, 512:768],
                        start=st, stop=(not first) and last, perf_mode=dr,
                        skip_group_check=True,
                    )
                    nc.tensor.matmul(
                        psB[:, 256:512], wsl, dsl[:, dlo : dlo + 2, 768:QCOL],
                        start=False, stop=(not first) and last, perf_mode=dr,
                        skip_group_check=True,
                    )

            def emit_evac(psA, psB, q, o):
                # Fused evacuation: out = psum/128 + bias[o], to bf16.
                # Both banks on DVE: the scalar/sync queues stay dedicated
                # to load DMAs so PSUM recycling never stalls behind them.
                # Separate half-tiles so each store only waits on its own
                # evacuation and the two stores ride both HW rings in
                # parallel (loads are all enqueued already, so a store's
                # semaphore wait cannot delay any load).
                osl = slice(o * P, (o + 1) * P)
                bcol = bias_t[:, o : o + 1]
                c0 = q * QCOL
                osbA = op.tile([P, 512], mybir.dt.bfloat16, tag="osbA", name="osbA")
                nc.vector.tensor_scalar(
                    osbA[:], psA[:],
                    1.0 / SCALE, bcol,
                    mybir.AluOpType.mult, mybir.AluOpType.add,
                )
                nc.scalar.dma_start(out=outT[osl, c0 : c0 + 512], in_=osbA[:])
                osbB = op.tile([P, 512], mybir.dt.bfloat16, tag="osbB", name="osbB")
                nc.vector.tensor_scalar(
                    osbB[:], psB[:],
                    1.0 / SCALE, bcol,
                    mybir.AluOpType.mult, mybir.AluOpType.add,
                )
                nc.sync.dma_start(out=outT[osl, c0 + 512 : c0 + QCOL], in_=osbB[:])

            # Phase 1 — groups (q0, o=0..3), DR-first: their fp8 matmuls only
            # need w8+d8q0 (512 KiB), so the PE does real work while the bf16
            # weight/data tiles stream in; the bf16 part then runs k-outer
            # across the four groups, matching DMA arrival order.
            ph1 = []
            for o in range(4):
                psA, psB = ps_first if o == 0 else (
                    pp.tile([P, 512], mybir.dt.float32, tag="pa", name="psA"),
                    pp.tile([P, 512], mybir.dt.float32, tag="pb", name="psB"),
                )
                ph1.append((psA, psB))
                emit_dr(psA, psB, 0, o, first=True)
            for k in range(KB):
                for o in range(4):
                    psA, psB = ph1[o]
                    lhsT = w_t[k][:, o * P : (o + 1) * P]
                    nc.tensor.matmul(
                        psA[:], lhsT, d_t[k][0][:, 0:512],
                        start=False, stop=(k == KB - 1),
                    )
                    nc.tensor.matmul(
                        psB[:], lhsT, d_t[k][0][:, 512:QCOL],
                        start=False, stop=(k == KB - 1),
                    )
            for o in range(4):
                emit_evac(ph1[o][0], ph1[o][1], 0, o)

            # Phase 2 — everything else in normal order (bf16 k-major, DR
            # tail) since all operands are SBUF-resident by then.
            for q in range(NQ):
                for o in range(4 if q == 0 else 0, NO):
                    psA = pp.tile([P, 512], mybir.dt.float32, tag="pa", name="psA")
                    psB = pp.tile([P, 512], mybir.dt.float32, tag="pb", name="psB")
                    for k in range(KB if q < 2 else KB - 2):
                        lhsT = w_t[k][:, o * P : (o + 1) * P]
                        nc.tensor.matmul(
                            psA[:], lhsT, d_t[k][q][:, 0:512],
                            start=(k == 0), stop=False,
                        )
                        nc.tensor.matmul(
                            psB[:], lhsT, d_t[k][q][:, 512:QCOL],
                            start=(k == 0), stop=False,
                        )
                    emit_dr(psA, psB, q, o, first=False)
                    emit_evac(psA, psB, q, o)

    nc.compile()
    return nc


def _get_nc():
    if "nc" not in _CACHE:
        _CACHE["nc"] = _build()
    return _CACHE["nc"]


def _prep_weights(W, b):
    W = np.asarray(W, dtype=np.float32)
    b = np.asarray(b, dtype=np.float32)
    Ws = W * SCALE
    # wT[k, p, o] = W[o, k*128+p] * 128  (bf16)
    wT = np.ascontiguousarray(
        Ws[:, : KB * P].T.reshape(KB, P, OUT_DIM).astype(BF)
    )
    # w8lo[p, i, o] = e4m3(W[o, 512 + i*128 + p] * 128)  (k-blocks 4,5)
    # w8hi[p, i, o] = e4m3(W[o, 768 + i*128 + p] * 128)  (k-blocks 6,7)
    w8lo = np.ascontiguousarray(
        Ws[:, 4 * P : 6 * P].T.reshape(2, P, OUT_DIM).transpose(1, 0, 2).astype(E4)
    )
    w8hi = np.ascontiguousarray(
        Ws[:, 6 * P :].T.reshape(2, P, OUT_DIM).transpose(1, 0, 2).astype(E4)
    )
    bias2 = np.ascontiguousarray(b.reshape(NO, P).T)  # [128, 8] f32
    wmup = np.zeros((P, 256), dtype=BF)
    return wT, w8lo, w8hi, bias2, wmup


def _prep_inputs(data, W, b):
    data = np.asarray(data, dtype=np.float32)
    wT, w8lo, w8hi, bias2, wmup = _prep_weights(W, b)
    in_maps = []
    for c in range(N_CORES):
        shard = data[c * SHARD : (c + 1) * SHARD]  # [4096, 1024] f32
        # dT[k, p, b] = bf16(shard[b, k*128+p])
        dTc = np.ascontiguousarray(
            shard[:, : KB * P].T.reshape(KB, P, SHARD).astype(BF)
        )
        # d8a[q, p, i, j] = e4m3(shard[q*1024+j, 768 + i*128 + p]), q = 0,1
        d8at = shard[: 2 * QCOL, 6 * P :].T.reshape(2, P, 2, QCOL)
        d8ac = np.ascontiguousarray(d8at.transpose(2, 1, 0, 3).astype(E4))
        # d8b[q, p, i, j] = e4m3(shard[(q+2)*1024+j, 512 + i*128 + p]), q = 0,1
        d8bt = shard[2 * QCOL :, 4 * P :].T.reshape(4, P, 2, QCOL)
        d8bc = np.ascontiguousarray(d8bt.transpose(2, 1, 0, 3).astype(E4))
        in_maps.append(
            {"dT": dTc, "d8a": d8ac, "d8b": d8bc, "wT": wT, "w8lo": w8lo,
             "w8hi": w8hi, "biasb": bias2, "wmup": wmup}
        )
    return in_maps


def _run(data, W, b, trace=False, **trace_kw):
    nc = _get_nc()
    in_maps = _prep_inputs(data, W, b)
    res = run_bass_kernel_spmd(
        nc, in_maps, list(range(N_CORES)), trace=trace, **trace_kw
    )
    out = np.concatenate(
        [
            np.asarray(res.results[c]["outT"]).T.astype(np.float32)
            for c in range(N_CORES)
        ],
        axis=0,
    )
    return out, res


def kernel(**inputs) -> np.ndarray:
    out, _ = _run(inputs["data"], inputs["W"], inputs["b"])
    return out
